# revision 28
# baseline (speedup 1.0000x reference)
"""Trainium2 Bass kernel for the STU (spectral transform unit) dense-transformer block.

Algorithm (validated against the jax reference in fp64 numpy):
  The FFT causal conv is rewritten as a block-Toeplitz matmul. For each of the
  K=16 filters and each sign branch (the alternating-sign branch folds into the
  filter taps: T^-[s,s'] = phi[s-s'] * (-1)^(s-s')), the causal conv is
    U_br = T_br @ u,  T_br block-Toeplitz with 16 distinct 128x128 blocks.
  sigma^(1/4) folds into the taps. The (k,i)->d projection contracts U with
  M_phi_{plus,minus}; the KU=3 autoregressive taps are shifted-u projections
  with M_u. MLP is a standard gated MLP.

Sharding (8 cores, no cross-core communication, host-side reduce between two
uniform SPMD programs):
  Phase 1: filter-branch-parallel. Core c computes conv + projection for its 4
           of the 32 (k, sign) branches over the full (B, SL): partial spectral.
  Host:    x1 = x + sum_c partial_c
  Phase 2: row-parallel. Core c owns 512 of the 4096 (b, s) rows: adds the AR
           term and computes the gated MLP + residual for its rows.

Precision: the conv runs in fp8 (output magnitude ~0.05 -> noise negligible);
phase 2 (AR + MLP) runs in bf16 which keeps the end-to-end error at ~3e-3
scale-relative (measured) against the 2e-2 harness gate.

Schedule notes (all targets are the InstructionCostModel timeline):
 - DMA is a serial ~360GB/s resource; transfers are emitted in the order
   compute needs them (x row-blocks and tw delta-chunks interleaved, weights
   after first-use rows), which removes the 24us/30us startup stalls the
   v1 kernel had.
 - PSUM->SBUF drain copies round-robin across DVE/Act/Pool so no single
   engine's copy latency gates the PE.
 - Phase-1 projection iterates cp-outer so its first matmuls depend only on
   the first conv psum drains, not the last.
"""

import numpy as np
import ml_dtypes

import concourse.bacc as bacc
import concourse.tile as tile
from concourse import mybir
from concourse.bass_utils import run_bass_kernel_spmd  # noqa: F401 (debug path)
from concourse.masks import make_identity


class _SpmdRunner:
    """Cached-jit SPMD executor: trace/compile once, then repeat calls only
    pay input upload + execution (mirrors bass2jax.run_bass_via_pjrt).

    ``shared`` names inputs that are identical on every core: they are fed
    replicated (host uploads one copy) instead of 8x-concatenated."""

    def __init__(self, nc, shared=(), volatile=()):
        import jax
        import concourse.mybir as _mb
        from concourse.bass2jax import (
            install_neuronx_cc_hook, _bass_exec_p, partition_id_tensor,
        )
        from jax.experimental.shard_map import shard_map
        from jax.sharding import Mesh, PartitionSpec

        install_neuronx_cc_hook()
        self.nc = nc
        assert nc.dbg_addr is None
        pid_name = (nc.partition_id_tensor.name
                    if nc.partition_id_tensor is not None else None)
        in_names, out_names, out_avals = [], [], []
        for alloc in nc.m.functions[0].allocations:
            if not isinstance(alloc, mybir.MemoryLocationSet):
                continue
            name = alloc.memorylocations[0].name
            if alloc.kind == "ExternalInput":
                if name != pid_name:
                    in_names.append(name)
            elif alloc.kind == "ExternalOutput":
                out_names.append(name)
                out_avals.append(jax.core.ShapedArray(
                    tuple(alloc.tensor_shape), mybir.dt.np(alloc.dtype)))
        self.in_names, self.out_names, self.out_avals = in_names, out_names, out_avals
        self.shared = frozenset(shared)
        self.volatile = frozenset(volatile)
        self._dev_cache = {}
        n_params = len(in_names)
        all_names = tuple(in_names + out_names)
        if pid_name is not None:
            all_names = all_names + (pid_name,)

        def _body(*args):
            args = list(args)
            if pid_name is not None:
                args.append(partition_id_tensor())
            return tuple(_bass_exec_p.bind(
                *args,
                out_avals=tuple(out_avals),
                in_names=all_names,
                out_names=tuple(out_names),
                lowering_input_output_aliases=(),
                sim_require_finite=True,
                sim_require_nnan=True,
                nc=nc,
            ))

        import jax.numpy as jnp
        from jax.sharding import NamedSharding
        devices = jax.devices()[:NCORES]
        mesh = Mesh(np.asarray(devices), ("core",))
        rep = PartitionSpec()
        core = PartitionSpec("core")
        in_specs = tuple(
            rep if nm in self.shared else core for nm in in_names
        ) + (core,) * len(out_names)
        out_specs = (core,) * len(out_names)
        donate = tuple(range(n_params, n_params + len(out_names)))
        self._fn = jax.jit(
            shard_map(_body, mesh=mesh, in_specs=in_specs, out_specs=out_specs,
                      check_rep=False),
            donate_argnums=donate, keep_unused=True,
        )
        self._zeros_fn = jax.jit(
            lambda: tuple(
                jnp.zeros((NCORES * a.shape[0], *a.shape[1:]), a.dtype)
                for a in out_avals
            ),
            out_shardings=tuple(
                NamedSharding(mesh, core) for _ in out_avals
            ),
        )
        self._shardings = {
            nm: NamedSharding(mesh, rep if nm in self.shared else core)
            for nm in in_names
        }

    def prep(self, in_maps):
        import hashlib
        import jax
        ins = []
        for nm in self.in_names:
            if nm in self.shared:
                arr = np.ascontiguousarray(in_maps[0][nm])
            else:
                arr = np.concatenate(
                    [np.asarray(in_maps[c][nm]) for c in range(NCORES)], axis=0)
            if nm in self.volatile:
                ins.append(arr)
                continue
            key = (nm, hashlib.md5(arr.tobytes()).hexdigest())
            dev = self._dev_cache.get(key)
            if dev is None:
                self._dev_cache.clear() if len(self._dev_cache) > 32 else None
                dev = jax.device_put(arr, self._shardings[nm])
                self._dev_cache[key] = dev
            ins.append(dev)
        return ins

    def run_prepped(self, ins):
        return self._fn(*ins, *self._zeros_fn())

    def __call__(self, in_maps):
        out_arrs = self.run_prepped(self.prep(in_maps))
        return [
            {nm: np.asarray(out_arrs[i]).reshape(NCORES, *self.out_avals[i].shape)[c]
             for i, nm in enumerate(self.out_names)}
            for c in range(NCORES)
        ]

BF16NP = ml_dtypes.bfloat16
FP8NP = ml_dtypes.float8_e4m3
TAP_SCALE = 1024.0
UT_SCALE = 32.0      # psum (TAP_SCALE*U) -> fp8 ut tiles scale factor: 32/1024
W_SCALE = 16.0       # projection weights scaled by 16 for fp8 range
SP_SCALE = UT_SCALE * W_SCALE  # spectral psum carries 32*16 = 512x
F32 = mybir.dt.float32
F32R = mybir.dt.float32r
F16 = mybir.dt.float16
BF = mybir.dt.bfloat16
FP8 = mybir.dt.float8e4

B, SL, D, K, KU = 2, 2048, 768, 16, 3
NFFT, EPS, P, H = 4096, 1e-5, 128, 3072
NB = SL // P            # 16 seq blocks
DC = D // P             # 6 d-chunks
NBR = 2 * K             # 32 conv branches
NCORES = 8
BPC = NBR // NCORES     # 4 branches per core
RPC = (B * SL) // NCORES  # 512 rows per core
MB = RPC // P           # 4 row blocks per core in phase 2
JC = H // P             # 24 hidden chunks
F1 = 512                # free-dim split of D=768 into 512+256
DR = mybir.MatmulPerfMode.DoubleRow

_cache: dict = {}


def _build_phase1():
    nc = bacc.Bacc("TRN2", target_bir_lowering=False, debug=False, num_devices=NCORES)
    xb = nc.dram_tensor("xb", (B, SL, D), BF, kind="ExternalInput").ap()
    tw = nc.dram_tensor("tw", (NB, P, 2, BPC * P), FP8, kind="ExternalInput").ap()
    wt = nc.dram_tensor("wt", (BPC, DC // 2, P, 2, D), FP8, kind="ExternalInput").ap()
    sp = nc.dram_tensor("sp", (B, SL, D), F16, kind="ExternalOutput").ap()

    with tile.TileContext(nc) as tc:
        with (
            tc.tile_pool(name="const", bufs=1) as const_pool,
            tc.tile_pool(name="ubuf", bufs=1) as ubuf_pool,
            tc.tile_pool(name="work", bufs=3) as work,
            tc.tile_pool(name="drain", bufs=4) as drain_pool,
            tc.tile_pool(name="spill", bufs=3) as spill_pool,
            tc.tile_pool(name="psum_u", bufs=4, space="PSUM") as psum_u_pool,
            tc.tile_pool(name="psum_sp", bufs=2, space="PSUM") as psum_sp_pool,
        ):
            eps_sb = const_pool.tile([P, 1], F32)
            nc.vector.memset(eps_sb, float(EPS))
            tw_sb = const_pool.tile([P, NB, 2, BPC * P], FP8)
            wt_sb = const_pool.tile([P, BPC, DC // 2, 2, D], FP8)

            # one persistent fp8 u tile per (b, J-pair): keeps the conv's
            # dependencies fine-grained (conv block I waits only on the pairs
            # it reads, not on all of u)
            u_t = [[ubuf_pool.tile([P, 2, D], FP8, name=f"u{b}_{jp}")
                    for jp in range(NB // 2)] for b in range(B)]

            def jprep(b, J):
                """x row-block DMA -> rmsnorm -> fp8 u pair-half.
                (rn1_w is folded into the projection weights host-side.)"""
                xt = work.tile([P, D], BF, name="xt")
                nc.sync.dma_start(xt, xb[b, J * P:(J + 1) * P, :])
                sq = work.tile([P, D], F32, name="sq")
                ms = work.tile([P, 1], F32, name="ms")
                nc.scalar.activation(
                    sq, xt, mybir.ActivationFunctionType.Square, accum_out=ms
                )
                nc.scalar.activation(
                    ms, ms, mybir.ActivationFunctionType.Sqrt,
                    bias=eps_sb, scale=1.0 / D,
                )
                nc.vector.reciprocal(ms, ms)
                nc.gpsimd.tensor_scalar_mul(
                    u_t[b][J // 2][:, J % 2, :], xt, ms
                )

            # PE warmup: dummy matmuls on a zero tile ramp the tensor
            # engine p-state while the first input blocks stream in, so the
            # first real conv matmuls run at full clock
            wz = const_pool.tile([P, 2, BPC * P], FP8, name="wz")
            nc.vector.memset(wz, 0.0)
            wps = psum_u_pool.tile([P, BPC * P], F32, name="psu")
            for _ in range(40):
                nc.tensor.matmul(wps, lhsT=wz[:, :, 0:P], rhs=wz,
                                 start=True, stop=True, perf_mode=DR)

            # prologue: just enough input for conv block I=0, weights after
            # the first row blocks they trail in the serial DMA queue
            nc.sync.dma_start(tw_sb[:, 0, :, :], tw[0])
            jprep(0, 0)
            jprep(0, 1)
            for j in range(2, 6):
                jprep(0, j)
            nc.sync.dma_start(wt_sb, wt.rearrange("b c p k f -> p b c k f"))
            next_j = [6, 0]

            drain_engines = (nc.vector, nc.scalar)

            def conv_block(b, I):
                ut_sb = drain_pool.tile([P, DC, BPC * P], FP8, name="ut")
                npair = I // 2 + 1
                for c in range(DC):
                    ps = psum_u_pool.tile([P, BPC * P], F32, name="psu")
                    for Jp in range(npair):
                        nc.tensor.matmul(
                            ps,
                            lhsT=u_t[b][Jp][:, :, c * P:(c + 1) * P],
                            rhs=tw_sb[:, I - 2 * Jp, :, :],
                            start=(Jp == 0),
                            stop=(Jp == npair - 1),
                            perf_mode=DR,
                        )
                    eng = drain_engines[c % 2]
                    if eng is nc.scalar:
                        nc.scalar.activation(
                            ut_sb[:, c, :], ps,
                            mybir.ActivationFunctionType.Copy,
                            scale=float(UT_SCALE / TAP_SCALE),
                        )
                    else:
                        eng.tensor_scalar_mul(
                            ut_sb[:, c, :], ps, float(UT_SCALE / TAP_SCALE)
                        )
                return ut_sb

            def proj_block(b, I, ut_sb):
                psp = psum_sp_pool.tile([P, D], F32, name="psp")
                n_mm = BPC * (DC // 2)
                i_mm = 0
                for cp in range(DC // 2):
                    for br in range(BPC):
                        st = i_mm == 0
                        fin = i_mm == n_mm - 1
                        lh = ut_sb[:, 2 * cp:2 * cp + 2, br * P:(br + 1) * P]
                        nc.tensor.matmul(
                            psp[:, 0:F1], lhsT=lh,
                            rhs=wt_sb[:, br, cp, :, 0:F1],
                            start=st, stop=fin, perf_mode=DR,
                        )
                        nc.tensor.matmul(
                            psp[:, F1:D], lhsT=lh,
                            rhs=wt_sb[:, br, cp, :, F1:D],
                            start=st, stop=fin, perf_mode=DR,
                        )
                        i_mm += 1
                sp_t = spill_pool.tile([P, D], F16, name="spt")
                last = b == B - 1 and I == NB - 1
                if last:
                    nc.scalar.activation(
                        sp_t[:, 0:F1], psp[:, 0:F1],
                        mybir.ActivationFunctionType.Copy,
                        scale=float(1.0 / SP_SCALE),
                    )
                    nc.sync.dma_start(
                        sp[b, I * P:(I + 1) * P, 0:F1], sp_t[:, 0:F1])
                    nc.vector.tensor_scalar_mul(
                        sp_t[:, F1:D], psp[:, F1:D], float(1.0 / SP_SCALE))
                    nc.sync.dma_start(
                        sp[b, I * P:(I + 1) * P, F1:D], sp_t[:, F1:D])
                elif I % 2 == 0:
                    nc.scalar.activation(
                        sp_t, psp, mybir.ActivationFunctionType.Copy,
                        scale=float(1.0 / SP_SCALE),
                    )
                    nc.sync.dma_start(sp[b, I * P:(I + 1) * P, :], sp_t)
                else:
                    nc.vector.tensor_scalar_mul(sp_t, psp, float(1.0 / SP_SCALE))
                    nc.sync.dma_start(sp[b, I * P:(I + 1) * P, :], sp_t)

            # software pipeline: emit the next block's conv before proj(I)
            # so the PE covers the psum-drain latency of block I; the first
            # (short) blocks keep two convs in flight
            from collections import deque
            pend = deque()
            for b in range(B):
                for I in range(NB):
                    # pace the DMA queue: tw delta-chunk I+1 and the u row
                    # blocks the next conv iterations will read
                    if b == 0 and I + 1 < NB:
                        nc.sync.dma_start(tw_sb[:, I + 1, :, :], tw[I + 1])
                    while next_j[b] <= min(I + 4, NB - 1):
                        jprep(b, next_j[b])
                        next_j[b] += 1
                    if b == 0 and I >= 8:
                        while next_j[1] <= min(2 * (I - 8) + 1, NB - 1):
                            jprep(1, next_j[1])
                            next_j[1] += 1

                    ut_sb = conv_block(b, I)
                    pend.append((b, I, ut_sb))
                    depth = 3 if (b == 0 and I < 4) else (2 if (b == 0 and I < 7) else 1)
                    while len(pend) > depth:
                        proj_block(*pend.popleft())
            while pend:
                proj_block(*pend.popleft())
    nc.compile()
    return nc


def _build_phase2():
    nc = bacc.Bacc("TRN2", target_bir_lowering=False, debug=False, num_devices=NCORES)
    xr = nc.dram_tensor("xr", (RPC + 2, D), BF, kind="ExternalInput").ap()
    x1r = nc.dram_tensor("x1r", (RPC, D), F32, kind="ExternalInput").ap()
    mut = nc.dram_tensor("mut", (KU, DC, P, D), BF, kind="ExternalInput").ap()
    fc1 = nc.dram_tensor("fc1", (JC, DC, P, 2, P), BF, kind="ExternalInput").ap()
    fc2 = nc.dram_tensor("fc2", (H, D), BF, kind="ExternalInput").ap()
    o = nc.dram_tensor("o", (RPC, D), F32, kind="ExternalOutput").ap()

    fc2_r = fc2.rearrange("(c p) d -> p c d", p=P)

    with tile.TileContext(nc) as tc:
        with (
            tc.tile_pool(name="const", bufs=1) as const_pool,
            tc.tile_pool(name="persist", bufs=1) as persist,
            tc.tile_pool(name="work", bufs=3) as work,
            tc.tile_pool(name="wstream", bufs=3) as wstream,
            tc.tile_pool(name="psum", bufs=4, space="PSUM") as psum_pool,
        ):
            ident = const_pool.tile([P, P], F32)
            make_identity(nc, ident)
            eps_sb = const_pool.tile([P, 1], F32)
            nc.vector.memset(eps_sb, float(EPS))

            ut_ext = persist.tile([P, DC, MB, P + 2], BF)
            x1p = persist.tile([P, MB, D], F32)
            xrows = persist.tile([P, MB, D], BF)
            x1rows = persist.tile([P, MB, D], F32)
            yt = persist.tile([P, DC, MB * P], BF)
            gt = persist.tile([P, JC, MB * P], BF)
            mut_sb = persist.tile([P, KU, DC, D], BF)
            fc2_sb = persist.tile([P, JC, D], BF)

            def rmsnorm_to(dst, src, rows):
                """dst = src / rms(src); the rmsnorm weight is folded into
                the downstream contraction weights host-side."""
                sq = work.tile([P, D], F32, name="sq")
                ms = work.tile([P, 1], F32, name="ms")
                nc.scalar.activation(
                    sq[:rows], src[:rows],
                    mybir.ActivationFunctionType.Square, accum_out=ms[:rows],
                )
                nc.scalar.activation(
                    ms[:rows], ms[:rows], mybir.ActivationFunctionType.Sqrt,
                    bias=eps_sb[:rows], scale=1.0 / D,
                )
                nc.vector.reciprocal(ms[:rows], ms[:rows])
                nc.gpsimd.tensor_scalar_mul(dst, src[:rows], ms[:rows])

            # DMA queue front: prefix rows, rn1, the 4 u row blocks, then the
            # mut taps (in per-tap-half chunks so AR starts on the first),
            # then x1 rows; fc1/fc2 stream later in the fws loop
            u_pre = persist.tile([2, D], F32)
            xp = work.tile([P, D], BF, name="xt")[:2]
            nc.sync.dma_start(xp, xr[0:2, :])
            for m in range(MB):
                nc.sync.dma_start(
                    xrows[:, m, :], xr[2 + m * P: 2 + (m + 1) * P, :])
            HC = DC // 2
            for t in range(KU):
                for h in range(2):
                    nc.sync.dma_start(
                        mut_sb[:, t, h * HC:(h + 1) * HC, :],
                        mut[t, h * HC:(h + 1) * HC].rearrange("c p d -> p c d"),
                    )
            for m in range(MB):
                nc.sync.dma_start(x1rows[:, m, :], x1r[m * P:(m + 1) * P, :])

            def psum_copy(dst, src_ps, idx):
                if idx % 2 == 0:
                    nc.vector.tensor_copy(dst, src_ps)
                else:
                    nc.scalar.activation(
                        dst, src_ps, mybir.ActivationFunctionType.Copy
                    )

            # ---- u^T tiles for the AR term (rmsnorm1 + PE transpose);
            # the 2-row prefix runs after the m blocks so it stays off the
            # critical path ----
            for m in range(MB):
                uo = work.tile([P, D], F32, name="uo")
                rmsnorm_to(uo, xrows[:, m, :], P)
                for c in range(DC):
                    pst = psum_pool.tile([P, D], F32, name="ps")[:, 0:P]
                    nc.tensor.transpose(pst, uo[:, c * P:(c + 1) * P], ident)
                    psum_copy(ut_ext[:, c, m, 2:P + 2], pst, c)
            rmsnorm_to(u_pre, xp, 2)
            for c in range(DC):
                pst2 = psum_pool.tile([P, D], F32, name="ps")[:, 0:P]
                nc.tensor.transpose(
                    pst2[:, 0:2], u_pre[:, c * P:(c + 1) * P], ident[0:2, 0:2]
                )
                nc.vector.tensor_copy(ut_ext[:, c, 0, 0:2], pst2[:, 0:2])
            for m in range(1, MB):
                for c in range(DC):
                    nc.gpsimd.tensor_copy(
                        ut_ext[:, c, m, 0:2], ut_ext[:, c, m - 1, P:P + 2]
                    )

            # ---- AR term: all 4 row-blocks accumulate per-(tap, d-half) in
            # mut arrival order so the psum groups start on the first chunk ----
            psa = [psum_pool.tile([P, D], F32, name="ps") for _ in range(MB)]
            for t in range(KU):
                for h in range(2):
                    for m in range(MB):
                        for c in range(h * HC, (h + 1) * HC):
                            st = t == 0 and c == 0
                            fin = t == KU - 1 and c == DC - 1
                            lh = ut_ext[:, c, m, 2 - t:P + 2 - t]
                            nc.tensor.matmul(
                                psa[m][:, 0:F1], lhsT=lh,
                                rhs=mut_sb[:, t, c, 0:F1], start=st, stop=fin,
                            )
                            nc.tensor.matmul(
                                psa[m][:, F1:D], lhsT=lh,
                                rhs=mut_sb[:, t, c, F1:D], start=st, stop=fin,
                            )
            for m in range(MB):
                nc.vector.tensor_tensor(
                    x1p[:, m, :], x1rows[:, m, :], psa[m], mybir.AluOpType.add
                )

            # fc1 weight chunks + fc2/mut resident weights, in first-use order
            fws = []
            for jc in range(JC):
                fw = wstream.tile([P, DC, 2, P], BF, name="fw")
                nc.sync.dma_start(fw, fc1[jc].rearrange("c p k f -> p c k f"))
                fws.append(fw)
                if jc == 1:
                    nc.sync.dma_start(fc2_sb, fc2_r)

            # ---- y = rmsnorm2(x1) transposed ----
            for m in range(MB):
                yf = work.tile([P, D], F32, name="uo")
                rmsnorm_to(yf, x1p[:, m, :], P)
                for c in range(DC):
                    pst = psum_pool.tile([P, D], F32, name="ps")[:, 0:P]
                    nc.tensor.transpose(pst, yf[:, c * P:(c + 1) * P], ident)
                    psum_copy(yt[:, c, m * P:(m + 1) * P], pst, c)

            # ---- fc1 + silu gate ----
            for jc in range(JC):
                ph1 = psum_pool.tile([P, D], F32, name="ps")[:, 0:F1]
                ph2 = psum_pool.tile([P, D], F32, name="ps")[:, 0:F1]
                for m in range(MB):
                    sl = slice(m * P, (m + 1) * P)
                    for c in range(DC):
                        nc.tensor.matmul(ph1[:, sl], lhsT=fws[jc][:, c, 0, :],
                                         rhs=yt[:, c, sl], start=c == 0,
                                         stop=c == DC - 1)
                        nc.tensor.matmul(ph2[:, sl], lhsT=fws[jc][:, c, 1, :],
                                         rhs=yt[:, c, sl], start=c == 0,
                                         stop=c == DC - 1)
                sact = work.tile([P, F1], F32, name="sact")
                nc.scalar.activation(sact, ph2, mybir.ActivationFunctionType.Silu)
                nc.vector.tensor_tensor(
                    gt[:, jc, :], ph1, sact, mybir.AluOpType.mult
                )

            # ---- fc2 + residual, m-outer so each row block's residual add
            # and output DMA overlap the next block's matmuls ----
            for m in range(MB):
                po = psum_pool.tile([P, D], F32, name="ps")
                for jc in range(JC):
                    st = jc == 0
                    fin = jc == JC - 1
                    nc.tensor.matmul(
                        po[:, 0:F1],
                        lhsT=gt[:, jc, m * P:(m + 1) * P],
                        rhs=fc2_sb[:, jc, 0:F1], start=st, stop=fin,
                    )
                    nc.tensor.matmul(
                        po[:, F1:D],
                        lhsT=gt[:, jc, m * P:(m + 1) * P],
                        rhs=fc2_sb[:, jc, F1:D], start=st, stop=fin,
                    )
                ot = work.tile([P, D], F32, name="ot")
                nc.vector.tensor_tensor(
                    ot[:, 0:F1], x1p[:, m, 0:F1], po[:, 0:F1],
                    mybir.AluOpType.add)
                nc.sync.dma_start(o[m * P:(m + 1) * P, 0:F1], ot[:, 0:F1])
                nc.vector.tensor_tensor(
                    ot[:, F1:D], x1p[:, m, F1:D], po[:, F1:D],
                    mybir.AluOpType.add)
                nc.sync.dma_start(o[m * P:(m + 1) * P, F1:D], ot[:, F1:D])
    nc.compile()
    return nc


def _host_prep(V, sigma, M_u, M_phi_plus, M_phi_minus, rn1):
    """Per-core weight tensors: Toeplitz tap blocks + projection matrices.
    rn1_w is folded into the projection's contraction axis (the rmsnorm
    weight commutes through the seq-dim conv)."""
    phi = np.fft.irfft(V.astype(np.complex128), n=NFFT, axis=0)[:SL]
    s4 = sigma.astype(np.float64) ** 0.25
    alt = (-1.0) ** np.arange(SL)

    taps = np.zeros((NBR, SL))
    Wb = np.zeros((NBR, D, D), np.float32)
    for k in range(K):
        taps[2 * k] = s4[k] * phi[:, k]
        taps[2 * k + 1] = s4[k] * phi[:, k] * alt
        Wb[2 * k] = M_phi_plus[k] * rn1[None, :]
        Wb[2 * k + 1] = M_phi_minus[k] * rn1[None, :]

    idx = np.arange(P)
    cmr = idx[None, :] - idx[:, None]       # [r, c] = c - r
    tw_cores = []
    wt_cores = []
    for core in range(NCORES):
        brs = range(core * BPC, (core + 1) * BPC)
        # tw[d0, :, ko, :] = T-block pair (delta=d0 for ko=0, delta=d0-1 for
        # ko=1, zeros for delta<0), taps scaled by TAP_SCALE for fp8 range
        tw = np.zeros((NB, P, 2, BPC * P), np.float32)
        wt = np.zeros((BPC, DC // 2, P, 2, D), np.float32)
        for bi, br in enumerate(brs):
            tsc = taps[br] * TAP_SCALE
            for d0 in range(NB):
                for ko in range(2):
                    d = d0 - ko
                    if d < 0:
                        continue
                    ii = d * P + cmr
                    blk = np.where(ii >= 0, tsc[np.clip(ii, 0, SL - 1)], 0.0)
                    tw[d0, :, ko, bi * P:(bi + 1) * P] = blk
            for cp in range(DC // 2):
                for ko in range(2):
                    c = 2 * cp + ko
                    # wt[bi, cp, i, ko, d] = Wb[br][d, c*P + i] * W_SCALE
                    wt[bi, cp, :, ko, :] = Wb[br][:, c * P:(c + 1) * P].T * W_SCALE
        tw_cores.append(tw.astype(FP8NP))
        wt_cores.append(wt.astype(FP8NP))
    return tw_cores, wt_cores


def kernel(x, V, sigma, M_u, M_phi_plus, M_phi_minus, rn1_w, rn2_w, fc1_w, fc2_w):
    x = np.ascontiguousarray(x, np.float32)
    if "p1" not in _cache:
        _cache["p1"] = _SpmdRunner(_build_phase1(), shared=("xb",), volatile=("xb",))
    if "p2" not in _cache:
        _cache["p2"] = _SpmdRunner(
            _build_phase2(), shared=("mut", "fc1", "fc2"),
            volatile=("xr", "x1r"))

    rn1 = np.ascontiguousarray(rn1_w, np.float32)
    rn2 = np.ascontiguousarray(rn2_w, np.float32)
    tw_cores, wt_cores = _host_prep(V, sigma, M_u, M_phi_plus, M_phi_minus, rn1)
    xb = x.astype(BF16NP)

    in_maps1 = [
        {"xb": xb, "tw": tw_cores[c], "wt": wt_cores[c]}
        for c in range(NCORES)
    ]
    r1 = _cache["p1"]
    sp_cat = r1.run_prepped(r1.prep(in_maps1))[0]
    if "reduce" not in _cache:
        import jax
        import jax.numpy as jnp
        from jax.sharding import NamedSharding, PartitionSpec
        sh = NamedSharding(r1._shardings["xb"].mesh, PartitionSpec())
        _cache["reduce"] = jax.jit(
            lambda spc, xx: xx + spc.reshape(NCORES, B, SL, D)
            .astype(jnp.float32).sum(0),
            out_shardings=sh,
        )
    x1 = np.asarray(_cache["reduce"](sp_cat, np.asarray(x)))

    # phase 2 inputs; rn1 folds into mut's contraction axis, rn2 into fc1's
    mut = np.zeros((KU, DC, P, D), np.float32)
    for t in range(KU):
        for c in range(DC):
            mut[t, c] = (M_u[t] * rn1[None, :])[:, c * P:(c + 1) * P].T
    mut = mut.astype(BF16NP)
    # fc1 pre-paired layout (JC, DC, P, 2, P): [..., 0, :] = y half column
    # block jc, [..., 1, :] = gate half column block jc
    f1s = np.ascontiguousarray(fc1_w, np.float32) * rn2[:, None]
    f1 = f1s.reshape(DC, P, 2, JC, P)
    fc1p = np.transpose(f1, (3, 0, 1, 2, 4)).astype(BF16NP)
    fc1p = np.ascontiguousarray(fc1p)
    fc2 = np.ascontiguousarray(fc2_w, np.float32).astype(BF16NP)

    x_rows = x.reshape(B * SL, D)
    x1_rows = x1.reshape(B * SL, D)
    in_maps2 = []
    for c in range(NCORES):
        r0 = c * RPC
        xr = np.zeros((RPC + 2, D), np.float32)
        xr[2:] = x_rows[r0:r0 + RPC]
        if r0 % SL != 0:
            xr[0:2] = x_rows[r0 - 2:r0]
        in_maps2.append({
            "xr": xr.astype(BF16NP),
            "x1r": np.ascontiguousarray(x1_rows[r0:r0 + RPC]),
            "mut": mut, "fc1": fc1p, "fc2": fc2,
        })
    res2 = _cache["p2"](in_maps2)
    out = np.concatenate(
        [res2[c]["o"] for c in range(NCORES)], axis=0
    ).reshape(B, SL, D)
    return out


# revision 30
# speedup vs baseline: 1.0300x; 1.0300x over previous
"""Trainium2 Bass kernel for the STU (spectral transform unit) dense-transformer block.

Algorithm (validated against the jax reference in fp64 numpy):
  The FFT causal conv is rewritten as a block-Toeplitz matmul. For each of the
  K=16 filters and each sign branch (the alternating-sign branch folds into the
  filter taps: T^-[s,s'] = phi[s-s'] * (-1)^(s-s')), the causal conv is
    U_br = T_br @ u,  T_br block-Toeplitz with 16 distinct 128x128 blocks.
  sigma^(1/4) folds into the taps. The (k,i)->d projection contracts U with
  M_phi_{plus,minus}; the KU=3 autoregressive taps are shifted-u projections
  with M_u. MLP is a standard gated MLP.

Sharding (8 cores, no cross-core communication, host-side reduce between two
uniform SPMD programs):
  Phase 1: filter-branch-parallel. Core c computes conv + projection for its 4
           of the 32 (k, sign) branches over the full (B, SL): partial spectral.
  Host:    x1 = x + sum_c partial_c
  Phase 2: row-parallel. Core c owns 512 of the 4096 (b, s) rows: adds the AR
           term and computes the gated MLP + residual for its rows.

Precision: the conv runs in fp8 (output magnitude ~0.05 -> noise negligible);
phase 2 (AR + MLP) runs in bf16 which keeps the end-to-end error at ~3e-3
scale-relative (measured) against the 2e-2 harness gate.

Schedule notes (all targets are the InstructionCostModel timeline):
 - DMA is a serial ~360GB/s resource; transfers are emitted in the order
   compute needs them (x row-blocks and tw delta-chunks interleaved, weights
   after first-use rows), which removes the 24us/30us startup stalls the
   v1 kernel had.
 - PSUM->SBUF drain copies round-robin across DVE/Act/Pool so no single
   engine's copy latency gates the PE.
 - Phase-1 projection iterates cp-outer so its first matmuls depend only on
   the first conv psum drains, not the last.
"""

import numpy as np
import ml_dtypes

import concourse.bacc as bacc
import concourse.tile as tile
from concourse import mybir
from concourse.bass_utils import run_bass_kernel_spmd  # noqa: F401 (debug path)
from concourse.masks import make_identity


class _SpmdRunner:
    """Cached-jit SPMD executor: trace/compile once, then repeat calls only
    pay input upload + execution (mirrors bass2jax.run_bass_via_pjrt).

    ``shared`` names inputs that are identical on every core: they are fed
    replicated (host uploads one copy) instead of 8x-concatenated."""

    def __init__(self, nc, shared=(), volatile=()):
        import jax
        import concourse.mybir as _mb
        from concourse.bass2jax import (
            install_neuronx_cc_hook, _bass_exec_p, partition_id_tensor,
        )
        from jax.experimental.shard_map import shard_map
        from jax.sharding import Mesh, PartitionSpec

        install_neuronx_cc_hook()
        self.nc = nc
        assert nc.dbg_addr is None
        pid_name = (nc.partition_id_tensor.name
                    if nc.partition_id_tensor is not None else None)
        in_names, out_names, out_avals = [], [], []
        for alloc in nc.m.functions[0].allocations:
            if not isinstance(alloc, mybir.MemoryLocationSet):
                continue
            name = alloc.memorylocations[0].name
            if alloc.kind == "ExternalInput":
                if name != pid_name:
                    in_names.append(name)
            elif alloc.kind == "ExternalOutput":
                out_names.append(name)
                out_avals.append(jax.core.ShapedArray(
                    tuple(alloc.tensor_shape), mybir.dt.np(alloc.dtype)))
        self.in_names, self.out_names, self.out_avals = in_names, out_names, out_avals
        self.shared = frozenset(shared)
        self.volatile = frozenset(volatile)
        self._dev_cache = {}
        n_params = len(in_names)
        all_names = tuple(in_names + out_names)
        if pid_name is not None:
            all_names = all_names + (pid_name,)

        def _body(*args):
            args = list(args)
            if pid_name is not None:
                args.append(partition_id_tensor())
            return tuple(_bass_exec_p.bind(
                *args,
                out_avals=tuple(out_avals),
                in_names=all_names,
                out_names=tuple(out_names),
                lowering_input_output_aliases=(),
                sim_require_finite=True,
                sim_require_nnan=True,
                nc=nc,
            ))

        import jax.numpy as jnp
        from jax.sharding import NamedSharding
        devices = jax.devices()[:NCORES]
        mesh = Mesh(np.asarray(devices), ("core",))
        rep = PartitionSpec()
        core = PartitionSpec("core")
        in_specs = tuple(
            rep if nm in self.shared else core for nm in in_names
        ) + (core,) * len(out_names)
        out_specs = (core,) * len(out_names)
        donate = tuple(range(n_params, n_params + len(out_names)))
        self._fn = jax.jit(
            shard_map(_body, mesh=mesh, in_specs=in_specs, out_specs=out_specs,
                      check_rep=False),
            donate_argnums=donate, keep_unused=True,
        )
        self._zeros_fn = jax.jit(
            lambda: tuple(
                jnp.zeros((NCORES * a.shape[0], *a.shape[1:]), a.dtype)
                for a in out_avals
            ),
            out_shardings=tuple(
                NamedSharding(mesh, core) for _ in out_avals
            ),
        )
        self._shardings = {
            nm: NamedSharding(mesh, rep if nm in self.shared else core)
            for nm in in_names
        }

    def prep(self, in_maps):
        import hashlib
        import jax
        ins = []
        for nm in self.in_names:
            if nm in self.shared:
                arr = np.ascontiguousarray(in_maps[0][nm])
            else:
                arr = np.concatenate(
                    [np.asarray(in_maps[c][nm]) for c in range(NCORES)], axis=0)
            if nm in self.volatile:
                ins.append(arr)
                continue
            key = (nm, hashlib.md5(arr.tobytes()).hexdigest())
            dev = self._dev_cache.get(key)
            if dev is None:
                self._dev_cache.clear() if len(self._dev_cache) > 32 else None
                dev = jax.device_put(arr, self._shardings[nm])
                self._dev_cache[key] = dev
            ins.append(dev)
        return ins

    def run_prepped(self, ins):
        return self._fn(*ins, *self._zeros_fn())

    def __call__(self, in_maps):
        out_arrs = self.run_prepped(self.prep(in_maps))
        return [
            {nm: np.asarray(out_arrs[i]).reshape(NCORES, *self.out_avals[i].shape)[c]
             for i, nm in enumerate(self.out_names)}
            for c in range(NCORES)
        ]

BF16NP = ml_dtypes.bfloat16
FP8NP = ml_dtypes.float8_e4m3
TAP_SCALE = 1024.0
UT_SCALE = 32.0      # psum (TAP_SCALE*U) -> fp8 ut tiles scale factor: 32/1024
W_SCALE = 16.0       # projection weights scaled by 16 for fp8 range
SP_SCALE = UT_SCALE * W_SCALE  # spectral psum carries 32*16 = 512x
MLP_SCALE = 16.0     # fc1 hi/lo fp8 weights carry 16x for fp8 range
F32 = mybir.dt.float32
F32R = mybir.dt.float32r
F16 = mybir.dt.float16
BF = mybir.dt.bfloat16
FP8 = mybir.dt.float8e4

B, SL, D, K, KU = 2, 2048, 768, 16, 3
NFFT, EPS, P, H = 4096, 1e-5, 128, 3072
NB = SL // P            # 16 seq blocks
DC = D // P             # 6 d-chunks
NBR = 2 * K             # 32 conv branches
NCORES = 8
BPC = NBR // NCORES     # 4 branches per core
RPC = (B * SL) // NCORES  # 512 rows per core
MB = RPC // P           # 4 row blocks per core in phase 2
JC = H // P             # 24 hidden chunks
F1 = 512                # free-dim split of D=768 into 512+256
DR = mybir.MatmulPerfMode.DoubleRow

_cache: dict = {}


def _build_phase1():
    nc = bacc.Bacc("TRN2", target_bir_lowering=False, debug=False, num_devices=NCORES)
    xb = nc.dram_tensor("xb", (B, SL, D), BF, kind="ExternalInput").ap()
    tw = nc.dram_tensor("tw", (NB, P, 2, BPC * P), FP8, kind="ExternalInput").ap()
    wt = nc.dram_tensor("wt", (BPC, DC // 2, P, 2, D), FP8, kind="ExternalInput").ap()
    sp = nc.dram_tensor("sp", (B, SL, D), F16, kind="ExternalOutput").ap()

    with tile.TileContext(nc) as tc:
        with (
            tc.tile_pool(name="const", bufs=1) as const_pool,
            tc.tile_pool(name="ubuf", bufs=1) as ubuf_pool,
            tc.tile_pool(name="work", bufs=3) as work,
            tc.tile_pool(name="drain", bufs=4) as drain_pool,
            tc.tile_pool(name="spill", bufs=3) as spill_pool,
            tc.tile_pool(name="psum_u", bufs=4, space="PSUM") as psum_u_pool,
            tc.tile_pool(name="psum_sp", bufs=2, space="PSUM") as psum_sp_pool,
        ):
            eps_sb = const_pool.tile([P, 1], F32)
            nc.vector.memset(eps_sb, float(EPS))
            tw_sb = const_pool.tile([P, NB, 2, BPC * P], FP8)
            wt_sb = const_pool.tile([P, BPC, DC // 2, 2, D], FP8)

            # one persistent fp8 u tile per (b, J-pair): keeps the conv's
            # dependencies fine-grained (conv block I waits only on the pairs
            # it reads, not on all of u)
            u_t = [[ubuf_pool.tile([P, 2, D], FP8, name=f"u{b}_{jp}")
                    for jp in range(NB // 2)] for b in range(B)]

            def jprep(b, J):
                """x row-block DMA -> rmsnorm -> fp8 u pair-half.
                (rn1_w is folded into the projection weights host-side.)"""
                xt = work.tile([P, D], BF, name="xt")
                nc.sync.dma_start(xt, xb[b, J * P:(J + 1) * P, :])
                sq = work.tile([P, D], F32, name="sq")
                ms = work.tile([P, 1], F32, name="ms")
                nc.scalar.activation(
                    sq, xt, mybir.ActivationFunctionType.Square, accum_out=ms
                )
                nc.scalar.activation(
                    ms, ms, mybir.ActivationFunctionType.Sqrt,
                    bias=eps_sb, scale=1.0 / D,
                )
                nc.vector.reciprocal(ms, ms)
                nc.gpsimd.tensor_scalar_mul(
                    u_t[b][J // 2][:, J % 2, :], xt, ms
                )

            # PE warmup: dummy matmuls on a zero tile ramp the tensor
            # engine p-state while the first input blocks stream in, so the
            # first real conv matmuls run at full clock
            wz = const_pool.tile([P, 2, BPC * P], FP8, name="wz")
            nc.vector.memset(wz, 0.0)
            wps = psum_u_pool.tile([P, BPC * P], F32, name="psu")
            for _ in range(40):
                nc.tensor.matmul(wps, lhsT=wz[:, :, 0:P], rhs=wz,
                                 start=True, stop=True, perf_mode=DR)

            # prologue: just enough input for conv block I=0, weights after
            # the first row blocks they trail in the serial DMA queue
            nc.sync.dma_start(tw_sb[:, 0, :, :], tw[0])
            jprep(0, 0)
            jprep(0, 1)
            for j in range(2, 6):
                jprep(0, j)
            nc.sync.dma_start(wt_sb, wt.rearrange("b c p k f -> p b c k f"))
            next_j = [6, 0]

            drain_engines = (nc.vector, nc.scalar)

            def conv_block(b, I):
                ut_sb = drain_pool.tile([P, DC, BPC * P], FP8, name="ut")
                npair = I // 2 + 1
                for c in range(DC):
                    ps = psum_u_pool.tile([P, BPC * P], F32, name="psu")
                    for Jp in range(npair):
                        nc.tensor.matmul(
                            ps,
                            lhsT=u_t[b][Jp][:, :, c * P:(c + 1) * P],
                            rhs=tw_sb[:, I - 2 * Jp, :, :],
                            start=(Jp == 0),
                            stop=(Jp == npair - 1),
                            perf_mode=DR,
                        )
                    eng = drain_engines[c % 2]
                    if eng is nc.scalar:
                        nc.scalar.activation(
                            ut_sb[:, c, :], ps,
                            mybir.ActivationFunctionType.Copy,
                            scale=float(UT_SCALE / TAP_SCALE),
                        )
                    else:
                        eng.tensor_scalar_mul(
                            ut_sb[:, c, :], ps, float(UT_SCALE / TAP_SCALE)
                        )
                return ut_sb

            def proj_block(b, I, ut_sb):
                psp = psum_sp_pool.tile([P, D], F32, name="psp")
                n_mm = BPC * (DC // 2)
                i_mm = 0
                for cp in range(DC // 2):
                    for br in range(BPC):
                        st = i_mm == 0
                        fin = i_mm == n_mm - 1
                        lh = ut_sb[:, 2 * cp:2 * cp + 2, br * P:(br + 1) * P]
                        nc.tensor.matmul(
                            psp[:, 0:F1], lhsT=lh,
                            rhs=wt_sb[:, br, cp, :, 0:F1],
                            start=st, stop=fin, perf_mode=DR,
                        )
                        nc.tensor.matmul(
                            psp[:, F1:D], lhsT=lh,
                            rhs=wt_sb[:, br, cp, :, F1:D],
                            start=st, stop=fin, perf_mode=DR,
                        )
                        i_mm += 1
                sp_t = spill_pool.tile([P, D], F16, name="spt")
                last = b == B - 1 and I == NB - 1
                if last:
                    nc.scalar.activation(
                        sp_t[:, 0:F1], psp[:, 0:F1],
                        mybir.ActivationFunctionType.Copy,
                        scale=float(1.0 / SP_SCALE),
                    )
                    nc.sync.dma_start(
                        sp[b, I * P:(I + 1) * P, 0:F1], sp_t[:, 0:F1])
                    nc.vector.tensor_scalar_mul(
                        sp_t[:, F1:D], psp[:, F1:D], float(1.0 / SP_SCALE))
                    nc.sync.dma_start(
                        sp[b, I * P:(I + 1) * P, F1:D], sp_t[:, F1:D])
                elif I % 2 == 0:
                    nc.scalar.activation(
                        sp_t, psp, mybir.ActivationFunctionType.Copy,
                        scale=float(1.0 / SP_SCALE),
                    )
                    nc.sync.dma_start(sp[b, I * P:(I + 1) * P, :], sp_t)
                else:
                    nc.vector.tensor_scalar_mul(sp_t, psp, float(1.0 / SP_SCALE))
                    nc.sync.dma_start(sp[b, I * P:(I + 1) * P, :], sp_t)

            # software pipeline: emit the next block's conv before proj(I)
            # so the PE covers the psum-drain latency of block I; the first
            # (short) blocks keep two convs in flight
            from collections import deque
            pend = deque()
            for b in range(B):
                for I in range(NB):
                    # pace the DMA queue: tw delta-chunk I+1 and the u row
                    # blocks the next conv iterations will read
                    if b == 0 and I + 1 < NB:
                        nc.sync.dma_start(tw_sb[:, I + 1, :, :], tw[I + 1])
                    while next_j[b] <= min(I + 4, NB - 1):
                        jprep(b, next_j[b])
                        next_j[b] += 1
                    if b == 0 and I >= 8:
                        while next_j[1] <= min(2 * (I - 8) + 1, NB - 1):
                            jprep(1, next_j[1])
                            next_j[1] += 1

                    ut_sb = conv_block(b, I)
                    pend.append((b, I, ut_sb))
                    depth = 3 if (b == 0 and I < 4) else (2 if (b == 0 and I < 7) else 1)
                    while len(pend) > depth:
                        proj_block(*pend.popleft())
            while pend:
                proj_block(*pend.popleft())
    nc.compile()
    return nc


def _build_phase2():
    nc = bacc.Bacc("TRN2", target_bir_lowering=False, debug=False, num_devices=NCORES)
    xr = nc.dram_tensor("xr", (RPC + 2, D), BF, kind="ExternalInput").ap()
    x1r = nc.dram_tensor("x1r", (RPC, D), F32, kind="ExternalInput").ap()
    mut = nc.dram_tensor("mut", (KU, DC, P, D), BF, kind="ExternalInput").ap()
    fc1 = nc.dram_tensor("fc1", (JC, DC, P, 2, 2, P), FP8, kind="ExternalInput").ap()
    fc2 = nc.dram_tensor("fc2", (H, D), BF, kind="ExternalInput").ap()
    o = nc.dram_tensor("o", (RPC, D), F32, kind="ExternalOutput").ap()

    fc2_r = fc2.rearrange("(c p) d -> p c d", p=P)

    with tile.TileContext(nc) as tc:
        with (
            tc.tile_pool(name="const", bufs=1) as const_pool,
            tc.tile_pool(name="persist", bufs=1) as persist,
            tc.tile_pool(name="work", bufs=3) as work,
            tc.tile_pool(name="wstream", bufs=3) as wstream,
            tc.tile_pool(name="psum", bufs=4, space="PSUM") as psum_pool,
        ):
            ident = const_pool.tile([P, P], F32)
            make_identity(nc, ident)
            eps_sb = const_pool.tile([P, 1], F32)
            nc.vector.memset(eps_sb, float(EPS))

            ut_ext = persist.tile([P, DC, MB, P + 2], BF)
            x1p = persist.tile([P, MB, D], F32)
            xrows = persist.tile([P, MB, D], BF)
            x1rows = persist.tile([P, MB, D], F32)
            yt = persist.tile([P, DC, 2, MB * P], FP8)
            gt = persist.tile([P, JC, MB * P], BF)
            mut_sb = persist.tile([P, KU, DC, D], BF)
            fc2_sb = persist.tile([P, JC, D], BF)

            def rmsnorm_to(dst, src, rows):
                """dst = src / rms(src); the rmsnorm weight is folded into
                the downstream contraction weights host-side."""
                sq = work.tile([P, D], F32, name="sq")
                ms = work.tile([P, 1], F32, name="ms")
                nc.scalar.activation(
                    sq[:rows], src[:rows],
                    mybir.ActivationFunctionType.Square, accum_out=ms[:rows],
                )
                nc.scalar.activation(
                    ms[:rows], ms[:rows], mybir.ActivationFunctionType.Sqrt,
                    bias=eps_sb[:rows], scale=1.0 / D,
                )
                nc.vector.reciprocal(ms[:rows], ms[:rows])
                nc.gpsimd.tensor_scalar_mul(dst, src[:rows], ms[:rows])

            # DMA queue front: prefix rows, rn1, the 4 u row blocks, then the
            # mut taps (in per-tap-half chunks so AR starts on the first),
            # then x1 rows; fc1/fc2 stream later in the fws loop
            u_pre = persist.tile([2, D], F32)
            xp = work.tile([P, D], BF, name="xt")[:2]
            nc.sync.dma_start(xp, xr[0:2, :])
            for m in range(MB):
                nc.sync.dma_start(
                    xrows[:, m, :], xr[2 + m * P: 2 + (m + 1) * P, :])
            HC = DC // 2
            for t in range(KU):
                for h in range(2):
                    nc.sync.dma_start(
                        mut_sb[:, t, h * HC:(h + 1) * HC, :],
                        mut[t, h * HC:(h + 1) * HC].rearrange("c p d -> p c d"),
                    )
            for m in range(MB):
                nc.sync.dma_start(x1rows[:, m, :], x1r[m * P:(m + 1) * P, :])

            def psum_copy(dst, src_ps, idx):
                if idx % 2 == 0:
                    nc.vector.tensor_copy(dst, src_ps)
                else:
                    nc.scalar.activation(
                        dst, src_ps, mybir.ActivationFunctionType.Copy
                    )

            # ---- u^T tiles for the AR term (rmsnorm1 + PE transpose);
            # the 2-row prefix runs after the m blocks so it stays off the
            # critical path ----
            for m in range(MB):
                uo = work.tile([P, D], F32, name="uo")
                rmsnorm_to(uo, xrows[:, m, :], P)
                for c in range(DC):
                    pst = psum_pool.tile([P, D], F32, name="ps")[:, 0:P]
                    nc.tensor.transpose(pst, uo[:, c * P:(c + 1) * P], ident)
                    psum_copy(ut_ext[:, c, m, 2:P + 2], pst, c)
            rmsnorm_to(u_pre, xp, 2)
            for c in range(DC):
                pst2 = psum_pool.tile([P, D], F32, name="ps")[:, 0:P]
                nc.tensor.transpose(
                    pst2[:, 0:2], u_pre[:, c * P:(c + 1) * P], ident[0:2, 0:2]
                )
                nc.vector.tensor_copy(ut_ext[:, c, 0, 0:2], pst2[:, 0:2])
            for m in range(1, MB):
                for c in range(DC):
                    nc.gpsimd.tensor_copy(
                        ut_ext[:, c, m, 0:2], ut_ext[:, c, m - 1, P:P + 2]
                    )

            # ---- AR term: all 4 row-blocks accumulate per-(tap, d-half) in
            # mut arrival order so the psum groups start on the first chunk ----
            psa = [psum_pool.tile([P, D], F32, name="ps") for _ in range(MB)]
            for t in range(KU):
                for h in range(2):
                    for m in range(MB):
                        for c in range(h * HC, (h + 1) * HC):
                            st = t == 0 and c == 0
                            fin = t == KU - 1 and c == DC - 1
                            lh = ut_ext[:, c, m, 2 - t:P + 2 - t]
                            nc.tensor.matmul(
                                psa[m][:, 0:F1], lhsT=lh,
                                rhs=mut_sb[:, t, c, 0:F1], start=st, stop=fin,
                            )
                            nc.tensor.matmul(
                                psa[m][:, F1:D], lhsT=lh,
                                rhs=mut_sb[:, t, c, F1:D], start=st, stop=fin,
                            )
            for m in range(MB):
                nc.vector.tensor_tensor(
                    x1p[:, m, :], x1rows[:, m, :], psa[m], mybir.AluOpType.add
                )

            # fc1 weight chunks + fc2/mut resident weights, in first-use order
            fws = []
            for jc in range(JC):
                fw = wstream.tile([P, DC, 2, 2, P], FP8, name="fw")
                nc.sync.dma_start(fw, fc1[jc].rearrange("c p q k f -> p c q k f"))
                fws.append(fw)
                if jc == 8:
                    nc.sync.dma_start(fc2_sb, fc2_r)

            # ---- y = rmsnorm2(x1) transposed ----
            for m in range(MB):
                yf = work.tile([P, D], F32, name="uo")
                rmsnorm_to(yf, x1p[:, m, :], P)
                for c in range(DC):
                    pst = psum_pool.tile([P, D], F32, name="ps")[:, 0:P]
                    nc.tensor.transpose(pst, yf[:, c * P:(c + 1) * P], ident)
                    sl = slice(m * P, (m + 1) * P)
                    psum_copy(yt[:, c, 0, sl], pst, c)
                    nc.vector.scalar_tensor_tensor(
                        yt[:, c, 1, sl], pst, 1.0, yt[:, c, 0, sl],
                        mybir.AluOpType.mult, mybir.AluOpType.subtract,
                    )

            # ---- fc1 + silu gate ----
            for jc in range(JC):
                ph1 = psum_pool.tile([P, D], F32, name="ps")[:, 0:F1]
                ph2 = psum_pool.tile([P, D], F32, name="ps")[:, 0:F1]
                for m in range(MB):
                    sl = slice(m * P, (m + 1) * P)
                    for hh, ph in ((0, ph1), (1, ph2)):
                        # y_hi @ w_hi, c-pairs packed in DoubleRow
                        for cp in range(DC // 2):
                            nc.tensor.matmul(
                                ph[:, sl],
                                lhsT=fws[jc][:, 2 * cp:2 * cp + 2, 1, hh, :],
                                rhs=yt[:, 2 * cp:2 * cp + 2, 0, sl],
                                start=cp == 0, stop=False, perf_mode=DR,
                            )
                        # cross terms w_lo@y_hi + w_hi@y_lo, DR-paired per c
                        for c in range(DC):
                            nc.tensor.matmul(
                                ph[:, sl],
                                lhsT=fws[jc][:, c, :, hh, :],
                                rhs=yt[:, c, :, sl],
                                start=False, stop=c == DC - 1, perf_mode=DR,
                            )
                sact = work.tile([P, F1], F32, name="sact")
                nc.scalar.activation(sact, ph2,
                                     mybir.ActivationFunctionType.Silu,
                                     scale=float(1.0 / MLP_SCALE))
                nc.vector.scalar_tensor_tensor(
                    gt[:, jc, :], ph1, float(1.0 / MLP_SCALE), sact,
                    mybir.AluOpType.mult, mybir.AluOpType.mult,
                )

            # ---- fc2 + residual, m-outer so each row block's residual add
            # and output DMA overlap the next block's matmuls ----
            for m in range(MB):
                po = psum_pool.tile([P, D], F32, name="ps")
                for jc in range(JC):
                    st = jc == 0
                    fin = jc == JC - 1
                    nc.tensor.matmul(
                        po[:, 0:F1],
                        lhsT=gt[:, jc, m * P:(m + 1) * P],
                        rhs=fc2_sb[:, jc, 0:F1], start=st, stop=fin,
                    )
                    nc.tensor.matmul(
                        po[:, F1:D],
                        lhsT=gt[:, jc, m * P:(m + 1) * P],
                        rhs=fc2_sb[:, jc, F1:D], start=st, stop=fin,
                    )
                ot = work.tile([P, D], F32, name="ot")
                nc.vector.tensor_tensor(
                    ot[:, 0:F1], x1p[:, m, 0:F1], po[:, 0:F1],
                    mybir.AluOpType.add)
                nc.sync.dma_start(o[m * P:(m + 1) * P, 0:F1], ot[:, 0:F1])
                nc.vector.tensor_tensor(
                    ot[:, F1:D], x1p[:, m, F1:D], po[:, F1:D],
                    mybir.AluOpType.add)
                nc.sync.dma_start(o[m * P:(m + 1) * P, F1:D], ot[:, F1:D])
    nc.compile()
    return nc


def _host_prep(V, sigma, M_u, M_phi_plus, M_phi_minus, rn1):
    """Per-core weight tensors: Toeplitz tap blocks + projection matrices.
    rn1_w is folded into the projection's contraction axis (the rmsnorm
    weight commutes through the seq-dim conv)."""
    phi = np.fft.irfft(V.astype(np.complex128), n=NFFT, axis=0)[:SL]
    s4 = sigma.astype(np.float64) ** 0.25
    alt = (-1.0) ** np.arange(SL)

    taps = np.zeros((NBR, SL))
    Wb = np.zeros((NBR, D, D), np.float32)
    for k in range(K):
        taps[2 * k] = s4[k] * phi[:, k]
        taps[2 * k + 1] = s4[k] * phi[:, k] * alt
        Wb[2 * k] = M_phi_plus[k] * rn1[None, :]
        Wb[2 * k + 1] = M_phi_minus[k] * rn1[None, :]

    idx = np.arange(P)
    cmr = idx[None, :] - idx[:, None]       # [r, c] = c - r
    tw_cores = []
    wt_cores = []
    for core in range(NCORES):
        brs = range(core * BPC, (core + 1) * BPC)
        # tw[d0, :, ko, :] = T-block pair (delta=d0 for ko=0, delta=d0-1 for
        # ko=1, zeros for delta<0), taps scaled by TAP_SCALE for fp8 range
        tw = np.zeros((NB, P, 2, BPC * P), np.float32)
        wt = np.zeros((BPC, DC // 2, P, 2, D), np.float32)
        for bi, br in enumerate(brs):
            tsc = taps[br] * TAP_SCALE
            for d0 in range(NB):
                for ko in range(2):
                    d = d0 - ko
                    if d < 0:
                        continue
                    ii = d * P + cmr
                    blk = np.where(ii >= 0, tsc[np.clip(ii, 0, SL - 1)], 0.0)
                    tw[d0, :, ko, bi * P:(bi + 1) * P] = blk
            for cp in range(DC // 2):
                for ko in range(2):
                    c = 2 * cp + ko
                    # wt[bi, cp, i, ko, d] = Wb[br][d, c*P + i] * W_SCALE
                    wt[bi, cp, :, ko, :] = Wb[br][:, c * P:(c + 1) * P].T * W_SCALE
        tw_cores.append(tw.astype(FP8NP))
        wt_cores.append(wt.astype(FP8NP))
    return tw_cores, wt_cores


def kernel(x, V, sigma, M_u, M_phi_plus, M_phi_minus, rn1_w, rn2_w, fc1_w, fc2_w):
    x = np.ascontiguousarray(x, np.float32)
    if "p1" not in _cache:
        _cache["p1"] = _SpmdRunner(_build_phase1(), shared=("xb",), volatile=("xb",))
    if "p2" not in _cache:
        _cache["p2"] = _SpmdRunner(
            _build_phase2(), shared=("mut", "fc1", "fc2"),
            volatile=("xr", "x1r"))

    rn1 = np.ascontiguousarray(rn1_w, np.float32)
    rn2 = np.ascontiguousarray(rn2_w, np.float32)
    tw_cores, wt_cores = _host_prep(V, sigma, M_u, M_phi_plus, M_phi_minus, rn1)
    xb = x.astype(BF16NP)

    in_maps1 = [
        {"xb": xb, "tw": tw_cores[c], "wt": wt_cores[c]}
        for c in range(NCORES)
    ]
    r1 = _cache["p1"]
    sp_cat = r1.run_prepped(r1.prep(in_maps1))[0]
    if "reduce" not in _cache:
        import jax
        import jax.numpy as jnp
        from jax.sharding import NamedSharding, PartitionSpec
        sh = NamedSharding(r1._shardings["xb"].mesh, PartitionSpec())
        _cache["reduce"] = jax.jit(
            lambda spc, xx: xx + spc.reshape(NCORES, B, SL, D)
            .astype(jnp.float32).sum(0),
            out_shardings=sh,
        )
    x1 = np.asarray(_cache["reduce"](sp_cat, np.asarray(x)))

    # phase 2 inputs; rn1 folds into mut's contraction axis, rn2 into fc1's
    mut = np.zeros((KU, DC, P, D), np.float32)
    for t in range(KU):
        for c in range(DC):
            mut[t, c] = (M_u[t] * rn1[None, :])[:, c * P:(c + 1) * P].T
    mut = mut.astype(BF16NP)
    # fc1 pre-paired layout (JC, DC, P, 2, P): [..., 0, :] = y half column
    # block jc, [..., 1, :] = gate half column block jc
    f1s = np.ascontiguousarray(fc1_w, np.float32) * rn2[:, None] * MLP_SCALE
    hi8 = f1s.astype(FP8NP)
    lo8 = (f1s - hi8.astype(np.float32)).astype(FP8NP)

    def _lay(a):
        return np.transpose(a.reshape(DC, P, 2, JC, P), (3, 0, 1, 2, 4))

    # q axis: 0 = lo, 1 = hi (cross-product DR pairs lo/hi against yt's hi/lo)
    fc1p = np.ascontiguousarray(np.stack([_lay(lo8), _lay(hi8)], axis=3))
    fc2 = np.ascontiguousarray(fc2_w, np.float32).astype(BF16NP)

    x_rows = x.reshape(B * SL, D)
    x1_rows = x1.reshape(B * SL, D)
    in_maps2 = []
    for c in range(NCORES):
        r0 = c * RPC
        xr = np.zeros((RPC + 2, D), np.float32)
        xr[2:] = x_rows[r0:r0 + RPC]
        if r0 % SL != 0:
            xr[0:2] = x_rows[r0 - 2:r0]
        in_maps2.append({
            "xr": xr.astype(BF16NP),
            "x1r": np.ascontiguousarray(x1_rows[r0:r0 + RPC]),
            "mut": mut, "fc1": fc1p, "fc2": fc2,
        })
    res2 = _cache["p2"](in_maps2)
    out = np.concatenate(
        [res2[c]["o"] for c in range(NCORES)], axis=0
    ).reshape(B, SL, D)
    return out
